# revision 9
# baseline (speedup 1.0000x reference)
# Multi-head causal attention (B=4, S=2048, D=1024, H=16) on 8 TRN2 NeuronCores.
#
# Sharding: batch x head-half. Core c handles batch b=c//2 and heads
# [8p, 8p+8) where p=c%2 (d-model slice [512p, 512p+512)). Every core runs
# the identical causal program: Q/K/V projections for its 8 heads over the
# full sequence, causal attention for all 4 query chunks of 512, and a
# PARTIAL output projection y_part = ctx_local @ woT[512p:512p+512] + b_o/2.
# The host unshards by summing the two partial outputs of each batch pair.
# No cross-core collectives; zero duplicated projection work; causal
# structure exploited exactly (diagonal tiles column-trimmed).
#
# Scores run in fp8e4m3 with DoubleRow perf mode (2x PE throughput): K/Q
# projections are cast straight to fp8 and partition-folded [128,S] ->
# [64,2,S] by an SBUF->SBUF DMA so each head's 64-dim contraction becomes
# 32 partitions x 2 rows. AV and projections stay bf16 (fp32 PSUM);
# softmax stats fp32. Attention uses transposed scores St[kk, q]:
#   St = Kt8.T @ Qt8 (DoubleRow; 4 heads packed in PE quadrants)
#   P = exp(St) (trimmed to the causal window on diagonal tiles), then
#   P[:, 0:128(r+1)] *= [zeros|tri] in place
#   OT[dv, q] += V_aug[kk, 65].T @ P  (ones column -> denominators free)
#   OT_norm = OT * reciprocal(bcast(denoms))
# Projection/output-projection PSUM groups are pumped from a filler queue
# INSIDE the attention step loop (borrowing St-pool slots) so the PE stays
# busy while the Scalar engine works through the exps.
import sys

if '/opt/trn_rl_repo' not in sys.path:
    sys.path.insert(0, '/opt/trn_rl_repo')

import numpy as np

B, S, D = 4, 2048, 1024
H, DK = 16, 64
NCORES = 8
SC = 512                 # query chunk
NHP = 4                  # local head-pairs per core (8 heads)
NCHUNK = S // SC         # 4 query chunks, chunk ci needs 4*(ci+1) kk tiles

_CACHE = {}


def _build_program():
    import contextlib

    import concourse.tile as tile
    from concourse import bacc, mybir

    F32 = mybir.dt.float32
    BF16 = mybir.dt.bfloat16
    FP8 = mybir.dt.float8e4
    DR = mybir.MatmulPerfMode.DoubleRow
    EXP = mybir.ActivationFunctionType.Exp

    nc = bacc.Bacc("TRN2", target_bir_lowering=False, debug=False,
                   num_devices=NCORES)

    xT_d = nc.dram_tensor("xT", [D, S], BF16, kind="ExternalInput")
    wqT_d = nc.dram_tensor("wqT", [D, 512], BF16, kind="ExternalInput")
    wkT_d = nc.dram_tensor("wkT", [D, 512], BF16, kind="ExternalInput")
    wvT_d = nc.dram_tensor("wvT", [D, 512], BF16, kind="ExternalInput")
    woT_d = nc.dram_tensor("woT", [512, D], BF16, kind="ExternalInput")
    bias_d = nc.dram_tensor("bias", [1, D], BF16, kind="ExternalInput")
    masks_d = nc.dram_tensor("masks", [128, 4 * 1024], BF16,
                             kind="ExternalInput")
    y_d = nc.dram_tensor("y", [S, D], F32, kind="ExternalOutput")

    with tile.TileContext(nc) as tc, contextlib.ExitStack() as ctx:
        smalls = ctx.enter_context(tc.tile_pool(name="smalls", bufs=1))
        p_OT = ctx.enter_context(tc.tile_pool(name="otp", bufs=1))
        p_K8 = ctx.enter_context(tc.tile_pool(name="k8p", bufs=1))
        p_Q8 = ctx.enter_context(tc.tile_pool(name="q8p", bufs=1))
        p_f8 = ctx.enter_context(tc.tile_pool(name="f8p", bufs=4))
        p_V = ctx.enter_context(tc.tile_pool(name="vp", bufs=1))
        p_mk = ctx.enter_context(tc.tile_pool(name="mk", bufs=1))
        p_w = ctx.enter_context(tc.tile_pool(name="wp", bufs=1))
        p_x = ctx.enter_context(tc.tile_pool(name="xp", bufs=4))
        p_rs = ctx.enter_context(tc.tile_pool(name="rs", bufs=2))
        p_bc = ctx.enter_context(tc.tile_pool(name="bcp", bufs=1))
        p_P = ctx.enter_context(tc.tile_pool(name="pp", bufs=8))
        p_yb = ctx.enter_context(tc.tile_pool(name="ybp", bufs=4))
        p_st = ctx.enter_context(tc.tile_pool(name="pst", bufs=2,
                                              space="PSUM"))
        p_av = ctx.enter_context(tc.tile_pool(name="pav", bufs=4,
                                              space="PSUM"))

        # ---------------- weights / constants DMA (one desc each) --------
        def fold_dram(ap, k):
            return ap.rearrange("(k p) s -> p k s", p=128)

        wk = p_w.tile([128, 8 * 512], BF16, tag="wk")
        wv = p_w.tile([128, 8 * 512], BF16, tag="wv")
        wq = p_w.tile([128, 8 * 512], BF16, tag="wq")
        nc.scalar.dma_start(wk[:].rearrange("p (k s) -> p k s", k=8),
                            fold_dram(wkT_d.ap(), 8))
        nc.scalar.dma_start(wv[:].rearrange("p (k s) -> p k s", k=8),
                            fold_dram(wvT_d.ap(), 8))
        nc.scalar.dma_start(wq[:].rearrange("p (k s) -> p k s", k=8),
                            fold_dram(wqT_d.ap(), 8))

        masks_sb = p_mk.tile([128, 4 * 1024], BF16, tag="masks")
        nc.gpsimd.dma_start(masks_sb[:], masks_d.ap())
        wo = p_w.tile([128, 4 * 1024], BF16, tag="wo")
        nc.gpsimd.dma_start(wo[:].rearrange("p (k s) -> p k s", k=4),
                            fold_dram(woT_d.ap(), 4))
        bias_sb = smalls.tile([1, D], BF16, tag="bias")
        nc.gpsimd.dma_start(bias_sb[:], bias_d.ap())
        biasbc = smalls.tile([128, D], BF16, tag="biasbc")
        nc.gpsimd.partition_broadcast(biasbc[:], bias_sb[:])

        onesf = smalls.tile([128, 128], F32, tag="onesf")
        nc.vector.memset(onesf[:], 1.0)

        OT = p_OT.tile([128, NHP * S], BF16, tag="OT")
        # fp8 K/Q: head (hp, hh) lives at partitions (hp%2)*64 + hh*32,
        # group g=hp//2 tile, [p, i, s] with head-dim d = 2p + i
        Kt8 = [p_K8.tile([128, 2 * S], FP8, tag=f"Kt8_{g}", name=f"Kt8_{g}")
               for g in range(2)]
        Qt8 = [p_Q8.tile([128, 2 * S], FP8, tag=f"Qt8_{g}", name=f"Qt8_{g}")
               for g in range(2)]
        Vsb = p_V.tile([128, 16 * 8 * 65], BF16, tag="Vsb")

        # ones columns of V_aug (16 s-tiles x 8 heads, one strided copy)
        nc.vector.tensor_copy(
            Vsb[:].rearrange("p (s h c) -> p s h c", s=16, c=65)
            [:, :, :, 64:65],
            onesf[:].rearrange("p (s h) -> p s h", s=16)[:, :, :, None])

        # pre-zero the P pool ring so mask-muls never read NaN garbage
        for i in range(8):
            pz = p_P.tile([128, 1024], BF16, tag="p", name=f"pz_{i}")
            nc.vector.memset(pz[:], 0.0)

        # x: all four s-chunks, one descriptor each
        xchs = []
        for sc in range(4):
            xch = p_x.tile([128, 8 * 512], BF16, tag="xch",
                           name=f"xch_{sc}")
            nc.sync.dma_start(
                xch[:].rearrange("p (k s) -> p k s", k=8),
                fold_dram(xT_d.ap()[:, sc * 512:(sc + 1) * 512], 8))
            xchs.append(xch)

        # ---------------- projection / out-proj group emitters -----------
        # each closure emits one PSUM group; `ps_pool` is whichever pool the
        # scheduler wants the group to borrow a slot from (p_av slots are
        # [128,512] f32 = 1 bank; p_st slots are [128,1024] = 2 banks and we
        # use the first half)
        def alloc_ps(ps_pool, name):
            if ps_pool is p_st:
                t = ps_pool.tile([128, 1024], F32, tag="st", name=name)
                return t[:, 0:512]
            return ps_pool.tile([128, 512], F32, tag="av", name=name)

        def k_group(sc, hp, kq, dst):
            w_t = wk if kq == 'k' else wq

            def emit(ps_pool):
                ps = alloc_ps(ps_pool, f"ps{kq}_{sc}_{hp}")
                for k in range(8):
                    nc.tensor.matmul(
                        ps[:],
                        w_t[:, k * 512 + hp * 128:k * 512 + (hp + 1) * 128],
                        xchs[sc][:, k * 512:(k + 1) * 512],
                        start=(k == 0), stop=(k == 7))
                f8 = p_f8.tile([128, 512], FP8, tag="f8")
                nc.vector.tensor_copy(f8[:], ps[:])
                # partition fold [128,512] -> [64,2,512] (d = 2p+i)
                nc.sync.dma_start(
                    dst[hp // 2][(hp % 2) * 64:(hp % 2) * 64 + 64]
                    .rearrange("p (i s) -> p i s", i=2)
                    [:, :, sc * 512:(sc + 1) * 512],
                    f8[:])
            return emit

        def v_group(st_g):
            def emit(ps_pool):
                ps = alloc_ps(ps_pool, f"psv_{st_g}")
                sti = st_g % 4
                for k in range(8):
                    nc.tensor.matmul(
                        ps[:],
                        xchs[st_g // 4][:, k * 512 + sti * 128:
                                        k * 512 + (sti + 1) * 128],
                        wv[:, k * 512:(k + 1) * 512],
                        start=(k == 0), stop=(k == 7))
                nc.vector.tensor_copy(
                    Vsb[:, st_g * 520:(st_g + 1) * 520]
                    .rearrange("p (h c) -> p h c", c=65)[:, :, 0:64],
                    ps[:].rearrange("p (h c) -> p h c", c=64))
            return emit

        def y_group(ci, qi, nc2):
            def emit(ps_pool):
                ps = alloc_ps(ps_pool, f"psy_{ci}_{qi}_{nc2}")
                for dc in range(4):
                    nc.tensor.matmul(
                        ps[:],
                        OT[:, dc * S + ci * SC + qi * 128:
                           dc * S + ci * SC + (qi + 1) * 128],
                        wo[:, dc * 1024 + nc2 * 512:
                           dc * 1024 + (nc2 + 1) * 512],
                        start=(dc == 0), stop=(dc == 3))
                yb = p_yb.tile([128, 512], F32, tag="yb")
                nc.vector.tensor_add(
                    yb[:], ps[:],
                    biasbc[:, nc2 * 512:(nc2 + 1) * 512])
                nc.sync.dma_start(
                    y_d.ap()[ci * SC + qi * 128:ci * SC + (qi + 1) * 128,
                             nc2 * 512:(nc2 + 1) * 512], yb[:])
            return emit

        def stage_groups(sc):
            gs = []
            for hp in range(NHP):
                gs.append(k_group(sc, hp, 'k', Kt8))
            for sti in range(4):
                gs.append(v_group(4 * sc + sti))
            for hp in range(NHP):
                gs.append(k_group(sc, hp, 'q', Qt8))
            return gs

        # filler queue: (deadline_stage, closure). Groups with
        # deadline_stage <= s must be flushed before chunk s starts.
        filler = []

        def pump(n, pool):
            for _ in range(min(n, len(filler))):
                _, emit = filler.pop(0)
                emit(pool)

        def flush_stage(s, pool):
            keep = []
            for dl, emit in filler:
                if dl <= s:
                    emit(pool)
                else:
                    keep.append((dl, emit))
            filler[:] = keep

        # stage 0 runs upfront (attention depends on it)
        for g in stage_groups(0):
            g(p_av)
        for dl, sc in ((1, 1), (2, 2), (3, 3)):
            filler.extend((dl, g) for g in stage_groups(sc))

        # ---------------- attention ----------------
        for ci in range(NCHUNK):
            cap = 4 * (ci + 1)
            for bl in range(2):
                av = [p_av.tile([128, 512], F32, tag="av",
                                name=f"av_{ci}_{bl}_{i}")
                      for i in range(4)]

                def emit_av(t, p_tiles, cap=cap, av=av, bl=bl):
                    for hp_i in range(2):
                        for hh in range(2):
                            hi = 2 * hp_i + hh
                            h = (2 * bl + hp_i) * 2 + hh
                            off = t * 520 + h * 65
                            nc.tensor.matmul(
                                av[hi][0:65, :],
                                Vsb[:, off:off + 65],
                                p_tiles[hp_i][:, hh * 512:(hh + 1) * 512],
                                start=(t == 0), stop=(t == cap - 1))

                pending = []
                for t in range(cap):
                    r = t - 4 * ci  # >=0 -> diagonal tile
                    q0 = 128 * r if r >= 0 else 0
                    p_cur = []
                    for hp_i in range(2):
                        hp = 2 * bl + hp_i
                        g, ph = hp // 2, hp % 2
                        st = p_st.tile([128, 1024], F32, tag="st")
                        for hh in range(2):
                            hb = ph * 64 + hh * 32
                            nc.tensor.matmul(
                                st[:, hh * 512 + q0:(hh + 1) * 512],
                                Kt8[g][hb:hb + 32]
                                .rearrange("p (i s) -> p i s", i=2)
                                [:, :, t * 128:(t + 1) * 128],
                                Qt8[g][hb:hb + 32]
                                .rearrange("p (i s) -> p i s", i=2)
                                [:, :, ci * SC + q0:(ci + 1) * SC],
                                start=True, stop=True,
                                perf_mode=DR, tile_position=(hb, 0))
                        p1 = p_P.tile([128, 1024], BF16, tag="p")
                        if r >= 1:
                            nc.scalar.activation(
                                p1[:].rearrange("p (h q) -> p h q", h=2)
                                [:, :, q0:512],
                                st[:].rearrange("p (h q) -> p h q", h=2)
                                [:, :, q0:512],
                                EXP)
                        else:
                            nc.scalar.activation(p1[:], st[:], EXP)
                        if r >= 0:
                            wm = q0 + 128
                            nc.vector.tensor_mul(
                                p1[:].rearrange("p (h q) -> p h q", h=2)
                                [:, :, 0:wm],
                                p1[:].rearrange("p (h q) -> p h q", h=2)
                                [:, :, 0:wm],
                                masks_sb[:, r * 1024:(r + 1) * 1024]
                                .rearrange("p (h q) -> p h q", h=2)
                                [:, :, 0:wm])
                        p_cur.append(p1)
                    pending.append((t, p_cur))
                    if len(pending) > 2:
                        tt, pp_t = pending.pop(0)
                        emit_av(tt, pp_t)
                    if t % 2 == 1:
                        pump(1, p_st)
                for tt, pp_t in pending:
                    emit_av(tt, pp_t)

                # normalize, one head-pair at a time
                for hp_i in range(2):
                    hp = 2 * bl + hp_i
                    rs = p_rs.tile([1, 1024], F32, tag="rs")
                    for hh in range(2):
                        hi = 2 * hp_i + hh
                        nc.vector.tensor_copy(
                            rs[0:1, hh * 512:hh * 512 + 512],
                            av[hi][64:65, :])
                    bc = p_bc.tile([128, 1024], F32, tag="bc")
                    nc.gpsimd.partition_broadcast(bc[:], rs[:])
                    rbc = p_bc.tile([128, 1024], F32, tag="rbc")
                    scr = p_bc.tile([128, 1024], F32, tag="scr")
                    nc.vector.reciprocal_approx_accurate(
                        rbc[:], bc[:], scratch=scr[:])
                    for hh in range(2):
                        hi = 2 * hp_i + hh
                        r0 = 64 * hh
                        nc.vector.tensor_mul(
                            OT[r0:r0 + 64,
                               hp * S + ci * SC:hp * S + (ci + 1) * SC],
                            av[hi][0:64, :],
                            rbc[r0:r0 + 64, hh * 512:hh * 512 + 512])

            # next chunk needs its K/Q/V projections complete
            flush_stage(ci + 1, p_av)
            # out-proj of this chunk becomes deadline-free filler work
            for qi in range(4):
                for nc2 in range(2):
                    filler.append((99, y_group(ci, qi, nc2)))
        flush_stage(99, p_av)

    nc.compile()
    return nc


def _get_program():
    if 'nc' not in _CACHE:
        _CACHE['nc'] = _build_program()
    return _CACHE['nc']


def _tri_masks():
    # masks[r] = [128, 2 x 512]: per hh half, [zeros(128r) | tri | ones]
    import ml_dtypes
    p = np.arange(128)[:, None]
    f = np.arange(512)[None, :]
    out = np.zeros((128, 4 * 1024), np.float32)
    for r in range(4):
        m = (p <= f - 128 * r).astype(np.float32)  # valid: key<=query
        out[:, r * 1024:r * 1024 + 512] = m
        out[:, r * 1024 + 512:(r + 1) * 1024] = m
    return out.astype(ml_dtypes.bfloat16)


def kernel(x, w_q, w_k, w_v, w_o, b_o):
    import ml_dtypes
    from concourse.bass_utils import run_bass_kernel_spmd

    BF = ml_dtypes.bfloat16
    x = np.asarray(x, dtype=np.float32)
    nc = _get_program()

    scale = np.float32(1.0 / np.sqrt(DK))
    wqT = np.ascontiguousarray(np.asarray(w_q, np.float32).T * scale)
    wkT = np.ascontiguousarray(np.asarray(w_k, np.float32).T)
    wvT = np.ascontiguousarray(np.asarray(w_v, np.float32).T)
    woT = np.ascontiguousarray(np.asarray(w_o, np.float32).T)
    bias_half = (np.asarray(b_o, np.float32) * 0.5)[None, :]
    masks = _tri_masks()

    xTs = [np.ascontiguousarray(x[b].T).astype(BF) for b in range(B)]
    in_maps = []
    for c in range(NCORES):
        b, p = c // 2, c % 2
        sl = slice(p * 512, (p + 1) * 512)
        in_maps.append({
            "xT": xTs[b],
            "wqT": wqT[:, sl].astype(BF),
            "wkT": wkT[:, sl].astype(BF),
            "wvT": wvT[:, sl].astype(BF),
            "woT": np.ascontiguousarray(woT[sl, :]).astype(BF),
            "bias": bias_half.astype(BF),
            "masks": masks,
        })

    res = run_bass_kernel_spmd(nc, in_maps, core_ids=list(range(NCORES)),
                               trace=_CACHE.get('trace', False),
                               tmpdir=_CACHE.get('tmpdir'))
    _CACHE['last_res'] = res

    y = np.empty((B, S, D), dtype=np.float32)
    for b in range(B):
        y[b] = res.results[2 * b]["y"] + res.results[2 * b + 1]["y"]
    return y
